# revision 13
# baseline (speedup 1.0000x reference)
"""AUROC surrogate loss on 8 TRN2 NeuronCores.

Reference (for s = sigmoid(y_pred), pos = y_true==1, neg = y_true==0):
    total = sum_{i in pos, j in neg} relu(1 - (s_i - s_j));  loss = total/(P*Q)

Because s in [0,1], s_i - s_j in [-1,1], so 1 - (s_i - s_j) >= 0 ALWAYS and the
relu never clips.  The O(N^2) pairwise sum is therefore exactly linear:
    total = P*Q - Q*S_pos + P*S_neg   =>   loss = 1 - S_pos/P + S_neg/Q
with S_pos/S_neg the sums of s over the positive/negative class.  This turns
the problem into an O(N) streaming reduction (memory-bound), which is what the
device computes.

Sharding strategy (data-parallel over the positive/negative axes, per the
problem hint): each core gets a contiguous 1/8 shard; within the shard the
host packs positive elements into whole 128-wide partitions first, then
negatives (slack filled with -40, whose sigmoid ~ 4e-18 is negligible).  The
device kernel per core is a single fused pass:
    DMA in [17,128] -> ScalarE sigmoid with accum_out (per-partition sums)
    -> DMA out [17,1]
The per-class partition split point is host bookkeeping; the host all-reduces
the 8 cores' partial sums and applies the closed-form formula (P and Q are
label counts, computed host-side as part of the unshard).

Engine program notes (hard-won on real silicon):
  * the ACT memzero is the first ScalarE op so walrus hoists the sigmoid PWP
    table load (~1.3us) to t~0 where it overlaps the input DMA
  * the output DMA is issued by ScalarE itself, in order after the
    activation+accum -- no cross-engine semaphore hop
  * no completion wait on the output DMA: the NEFF's multi-microsecond
    engine-teardown sequence runs after the module body, giving the 68-byte
    transfer ample time to land before execution completes (validated over
    many runs); this keeps ~1us of HWDGE completion latency off the
    critical path.

Robustness: the axon/PJRT device session occasionally reports a transient
NRT_EXEC_UNIT_UNRECOVERABLE on a cold execution, and the failure is sticky
for the whole process.  kernel() retries in-process first, then falls back to
a fresh subprocess (new device session), which reliably recovers.
"""

import os
import subprocess
import sys
import tempfile
import time

import numpy as np

N = 16384
N_CORES = 8
SHARD = N // N_CORES  # 2048
PARTS = 17  # ceil(pos/128) + ceil(neg/128) <= 17 for any split of 2048
F = 128
PAD = np.float32(-40.0)  # sigmoid(-40) ~ 4.25e-18

_NC_CACHE = {}


def build_nc():
    import concourse.bass as bass
    from concourse import mybir

    # Bass.__init__ registers four const tiles (gpsimd memsets) and closes with
    # an all-engine barrier; this kernel reads none of the consts, so skip the
    # barrier (defensively -- fall back to the stock constructor on any error).
    ctor_kw = dict(
        num_devices=N_CORES, enable_partition_id=False, monotonic_sem_count=0
    )
    try:
        orig_barrier = bass.Bass.all_engine_barrier
        bass.Bass.all_engine_barrier = lambda self, **kw: None
        try:
            nc = bass.Bass(**ctor_kw)
        finally:
            bass.Bass.all_engine_barrier = orig_barrier
    except Exception:  # noqa: BLE001
        nc = bass.Bass(**ctor_kw)
    yp = nc.dram_tensor("yp_packed", [PARTS, F], mybir.dt.float32, kind="ExternalInput")
    out = nc.dram_tensor("psums", [PARTS, 1], mybir.dt.float32, kind="ExternalOutput")

    with (
        nc.sbuf_tensor([PARTS, F], mybir.dt.float32) as ypt,
        nc.sbuf_tensor([PARTS, F], mybir.dt.float32) as s,
        nc.sbuf_tensor([PARTS, 1], mybir.dt.float32) as red,
        nc.sbuf_tensor([PARTS, 1], mybir.dt.float32) as bias,
        nc.semaphore() as sp,
        nc.semaphore() as so,
        nc.Block() as block,
    ):

        @block.scalar
        def _(scalar):
            # first ACT op: walrus inserts the PWP table load before it,
            # overlapping the input DMA; also zeroes the sigmoid bias AP
            scalar.memzero(bias[:])
            scalar.drain()  # bias write retired before the activation reads it
            scalar.wait_ge(sp, 16)
            scalar.activation(
                out=s[:],
                in_=ypt[:],
                func=mybir.ActivationFunctionType.Sigmoid,
                bias=bias[:],
                accum_out=red[:],  # per-partition sum of sigmoid
            )
            # No retirement barrier before the DMA trigger: the accumulator
            # write retires with the READ_ACCUMULATOR machine op (~280ns engine
            # tail), while the HWDGE descriptor generation the trigger kicks
            # off takes ~550ns before the DMA engine reads red -- a structural
            # margin validated over hundreds of executions.  An explicit
            # drain/sem here costs ~0.9us (serializes the ACT pipeline tail
            # against descriptor generation).
            scalar.dma_start(out=out.ap(), in_=red[:]).then_inc(so, 16)

        @block.sync
        def _(sync):
            sync.dma_start(out=ypt[:], in_=yp.ap()).then_inc(sp, 16)

    return nc


def get_nc():
    if "nc" not in _NC_CACHE:
        _NC_CACHE["nc"] = build_nc()
    return _NC_CACHE["nc"]


def _pack_shard(yp_shard, yt_shard):
    """Pack one core's shard: positives fill whole partitions first (padded),
    then negatives (padded).  Returns (packed [PARTS,F] f32, n_pos_partitions).
    """
    pos = yp_shard[yt_shard == 1]
    neg = yp_shard[yt_shard == 0]
    pos_parts = (len(pos) + F - 1) // F
    packed = np.full((PARTS, F), PAD, dtype=np.float32)
    flat = packed.reshape(-1)
    flat[: len(pos)] = pos
    flat[pos_parts * F : pos_parts * F + len(neg)] = neg
    return packed, pos_parts


def _run_device(packed_list):
    """Run the SPMD kernel; returns [N_CORES, PARTS] f32 per-partition sums."""
    from concourse import bass_utils

    in_maps = [{"yp_packed": p} for p in packed_list]
    nc = get_nc()
    res = bass_utils.run_bass_kernel_spmd(nc, in_maps, core_ids=list(range(N_CORES)))
    return np.stack([np.asarray(r["psums"]).reshape(-1) for r in res.results])


_CHILD_CODE = """
import numpy as np, sys
sys.path.insert(0, {kdir!r})
import importlib.util
spec = importlib.util.spec_from_file_location("_kernel_child", {kfile!r})
mod = importlib.util.module_from_spec(spec)
spec.loader.exec_module(mod)
d = np.load(sys.argv[1])
psums = mod._run_device([d[f"p{{i}}"] for i in range({ncores})])
np.save(sys.argv[2], psums)
"""


def _run_device_subprocess(packed_list):
    kfile = os.path.abspath(__file__)
    code = _CHILD_CODE.format(kdir=os.path.dirname(kfile), kfile=kfile, ncores=N_CORES)
    with tempfile.TemporaryDirectory() as td:
        inp = os.path.join(td, "in.npz")
        outp = os.path.join(td, "out.npy")
        np.savez(inp, **{f"p{i}": p for i, p in enumerate(packed_list)})
        env = dict(os.environ)
        env.pop("JAX_PLATFORMS", None)
        subprocess.run(
            [sys.executable, "-c", code, inp, outp],
            check=True,
            env=env,
            timeout=900,
            capture_output=True,
        )
        return np.load(outp)


def kernel(y_pred, y_true):
    y_pred = np.asarray(y_pred, dtype=np.float32).reshape(N)
    y_true = np.asarray(y_true, dtype=np.int32).reshape(N)

    packed_list = []
    pos_parts = []
    for i in range(N_CORES):
        sl = slice(i * SHARD, (i + 1) * SHARD)
        packed, pp = _pack_shard(y_pred[sl], y_true[sl])
        packed_list.append(packed)
        pos_parts.append(pp)

    psums = None
    last_exc = None
    try:
        psums = _run_device(packed_list)
    except Exception as e:  # noqa: BLE001
        # A failed execution (e.g. transient NRT_EXEC_UNIT_UNRECOVERABLE) is
        # sticky for this process's device session -- recover via fresh
        # subprocesses (new session) instead of retrying in-process.
        last_exc = e
        for attempt in range(3):
            try:
                psums = _run_device_subprocess(packed_list)
                break
            except Exception as e2:  # noqa: BLE001
                last_exc = e2
                time.sleep(2.0 + 3.0 * attempt)
    if psums is None:
        raise last_exc

    s_pos = 0.0
    s_neg = 0.0
    for pp, row in zip(pos_parts, psums):
        row = row.astype(np.float64)
        s_pos += row[:pp].sum()
        s_neg += row[pp:].sum()

    p_cnt = float((y_true == 1).sum())
    q_cnt = float((y_true == 0).sum())
    if p_cnt * q_cnt <= 0:
        return np.array(0.0, dtype=np.float32)
    loss = 1.0 - s_pos / p_cnt + s_neg / q_cnt
    return np.array(loss, dtype=np.float32)


# revision 14
# speedup vs baseline: 1.0554x; 1.0554x over previous
"""AUROC surrogate loss on 8 TRN2 NeuronCores.

Reference (for s = sigmoid(y_pred), pos = y_true==1, neg = y_true==0):
    total = sum_{i in pos, j in neg} relu(1 - (s_i - s_j));  loss = total/(P*Q)

Because s in [0,1], s_i - s_j in [-1,1], so 1 - (s_i - s_j) >= 0 ALWAYS and the
relu never clips.  The O(N^2) pairwise sum is therefore exactly linear:
    total = P*Q - Q*S_pos + P*S_neg   =>   loss = 1 - S_pos/P + S_neg/Q
with S_pos/S_neg the sums of s over the positive/negative class.  This turns
the problem into an O(N) streaming reduction (memory-bound), which is what the
device computes.

Sharding strategy (data-parallel over the positive/negative axes, per the
problem hint): each core gets a contiguous 1/8 shard; within the shard the
host packs positive elements into whole 128-wide partitions first, then
negatives (slack filled with -40, whose sigmoid ~ 4e-18 is negligible).  The
device kernel per core is a single fused pass:
    DMA in [17,128] -> ScalarE sigmoid with accum_out (per-partition sums)
    -> DMA out [17,1]
The per-class partition split point is host bookkeeping; the host all-reduces
the 8 cores' partial sums and applies the closed-form formula (P and Q are
label counts, computed host-side as part of the unshard).

Engine program notes (hard-won on real silicon):
  * the ACT memzero is the first ScalarE op so walrus hoists the sigmoid PWP
    table load (~1.3us) to t~0 where it overlaps the input DMA
  * the output DMA is issued by ScalarE itself, in order after the
    activation+accum -- no cross-engine semaphore hop
  * no completion wait on the output DMA: the NEFF's multi-microsecond
    engine-teardown sequence runs after the module body, giving the 68-byte
    transfer ample time to land before execution completes (validated over
    many runs); this keeps ~1us of HWDGE completion latency off the
    critical path.

Robustness: the axon/PJRT device session occasionally reports a transient
NRT_EXEC_UNIT_UNRECOVERABLE on a cold execution, and the failure is sticky
for the whole process.  kernel() retries in-process first, then falls back to
a fresh subprocess (new device session), which reliably recovers.
"""

import os
import subprocess
import sys
import tempfile
import time

import numpy as np

N = 16384
N_CORES = 8
SHARD = N // N_CORES  # 2048
PARTS = 17  # ceil(pos/128) + ceil(neg/128) <= 17 for any split of 2048
F = 128
PAD = np.float32(-40.0)  # sigmoid(-40) ~ 4.25e-18

_NC_CACHE = {}


def build_nc():
    import concourse.bass as bass
    from concourse import mybir

    # Bass.__init__ registers four const tiles (gpsimd memsets, ~0.4us of dead
    # stores this kernel never reads -- the BIR verifier flags them as
    # reader-less) and closes with an all-engine barrier.  Skip both during
    # construction: the NEFF gets smaller and the measured critical path
    # starts at real work (worth ~2.2us on silicon).  Defensive: fall back to
    # the stock constructor on any error.
    ctor_kw = dict(
        num_devices=N_CORES, enable_partition_id=False, monotonic_sem_count=0
    )
    try:
        orig_barrier = bass.Bass.all_engine_barrier
        memset_owner = None
        for klass in bass.BassGpSimd.__mro__:
            if "memset" in vars(klass):
                memset_owner = klass
                break
        orig_memset = memset_owner.memset
        bass.Bass.all_engine_barrier = lambda self, **kw: None
        memset_owner.memset = lambda self, ap, c: None
        try:
            nc = bass.Bass(**ctor_kw)
        finally:
            bass.Bass.all_engine_barrier = orig_barrier
            memset_owner.memset = orig_memset
    except Exception:  # noqa: BLE001
        nc = bass.Bass(**ctor_kw)
    yp = nc.dram_tensor("yp_packed", [PARTS, F], mybir.dt.float32, kind="ExternalInput")
    out = nc.dram_tensor("psums", [PARTS, 1], mybir.dt.float32, kind="ExternalOutput")

    with (
        nc.sbuf_tensor([PARTS, F], mybir.dt.float32) as ypt,
        nc.sbuf_tensor([PARTS, F], mybir.dt.float32) as s,
        nc.sbuf_tensor([PARTS, 1], mybir.dt.float32) as red,
        nc.sbuf_tensor([PARTS, 1], mybir.dt.float32) as bias,
        nc.semaphore() as sp,
        nc.semaphore() as so,
        nc.Block() as block,
    ):

        @block.scalar
        def _(scalar):
            # first ACT op: walrus inserts the PWP table load before it,
            # overlapping the input DMA; also zeroes the sigmoid bias AP
            scalar.memzero(bias[:])
            scalar.drain()  # bias write retired before the activation reads it
            scalar.wait_ge(sp, 16)
            scalar.activation(
                out=s[:],
                in_=ypt[:],
                func=mybir.ActivationFunctionType.Sigmoid,
                bias=bias[:],
                accum_out=red[:],  # per-partition sum of sigmoid
            )
            # No retirement barrier before the DMA trigger: the accumulator
            # write retires with the READ_ACCUMULATOR machine op (~280ns engine
            # tail), while the HWDGE descriptor generation the trigger kicks
            # off takes ~550ns before the DMA engine reads red -- a structural
            # margin validated over hundreds of executions.  An explicit
            # drain/sem here costs ~0.9us (serializes the ACT pipeline tail
            # against descriptor generation).
            scalar.dma_start(out=out.ap(), in_=red[:]).then_inc(so, 16)

        @block.sync
        def _(sync):
            sync.dma_start(out=ypt[:], in_=yp.ap()).then_inc(sp, 16)

    return nc


def get_nc():
    if "nc" not in _NC_CACHE:
        _NC_CACHE["nc"] = build_nc()
    return _NC_CACHE["nc"]


def _pack_shard(yp_shard, yt_shard):
    """Pack one core's shard: positives fill whole partitions first (padded),
    then negatives (padded).  Returns (packed [PARTS,F] f32, n_pos_partitions).
    """
    pos = yp_shard[yt_shard == 1]
    neg = yp_shard[yt_shard == 0]
    pos_parts = (len(pos) + F - 1) // F
    packed = np.full((PARTS, F), PAD, dtype=np.float32)
    flat = packed.reshape(-1)
    flat[: len(pos)] = pos
    flat[pos_parts * F : pos_parts * F + len(neg)] = neg
    return packed, pos_parts


def _run_device(packed_list):
    """Run the SPMD kernel; returns [N_CORES, PARTS] f32 per-partition sums."""
    from concourse import bass_utils

    in_maps = [{"yp_packed": p} for p in packed_list]
    nc = get_nc()
    res = bass_utils.run_bass_kernel_spmd(nc, in_maps, core_ids=list(range(N_CORES)))
    return np.stack([np.asarray(r["psums"]).reshape(-1) for r in res.results])


_CHILD_CODE = """
import numpy as np, sys
sys.path.insert(0, {kdir!r})
import importlib.util
spec = importlib.util.spec_from_file_location("_kernel_child", {kfile!r})
mod = importlib.util.module_from_spec(spec)
spec.loader.exec_module(mod)
d = np.load(sys.argv[1])
psums = mod._run_device([d[f"p{{i}}"] for i in range({ncores})])
np.save(sys.argv[2], psums)
"""


def _run_device_subprocess(packed_list):
    kfile = os.path.abspath(__file__)
    code = _CHILD_CODE.format(kdir=os.path.dirname(kfile), kfile=kfile, ncores=N_CORES)
    with tempfile.TemporaryDirectory() as td:
        inp = os.path.join(td, "in.npz")
        outp = os.path.join(td, "out.npy")
        np.savez(inp, **{f"p{i}": p for i, p in enumerate(packed_list)})
        env = dict(os.environ)
        env.pop("JAX_PLATFORMS", None)
        subprocess.run(
            [sys.executable, "-c", code, inp, outp],
            check=True,
            env=env,
            timeout=900,
            capture_output=True,
        )
        return np.load(outp)


def kernel(y_pred, y_true):
    y_pred = np.asarray(y_pred, dtype=np.float32).reshape(N)
    y_true = np.asarray(y_true, dtype=np.int32).reshape(N)

    packed_list = []
    pos_parts = []
    for i in range(N_CORES):
        sl = slice(i * SHARD, (i + 1) * SHARD)
        packed, pp = _pack_shard(y_pred[sl], y_true[sl])
        packed_list.append(packed)
        pos_parts.append(pp)

    psums = None
    last_exc = None
    try:
        psums = _run_device(packed_list)
    except Exception as e:  # noqa: BLE001
        # A failed execution (e.g. transient NRT_EXEC_UNIT_UNRECOVERABLE) is
        # sticky for this process's device session -- recover via fresh
        # subprocesses (new session) instead of retrying in-process.
        last_exc = e
        for attempt in range(3):
            try:
                psums = _run_device_subprocess(packed_list)
                break
            except Exception as e2:  # noqa: BLE001
                last_exc = e2
                time.sleep(2.0 + 3.0 * attempt)
    if psums is None:
        raise last_exc

    s_pos = 0.0
    s_neg = 0.0
    for pp, row in zip(pos_parts, psums):
        row = row.astype(np.float64)
        s_pos += row[:pp].sum()
        s_neg += row[pp:].sum()

    p_cnt = float((y_true == 1).sum())
    q_cnt = float((y_true == 0).sum())
    if p_cnt * q_cnt <= 0:
        return np.array(0.0, dtype=np.float32)
    loss = 1.0 - s_pos / p_cnt + s_neg / q_cnt
    return np.array(loss, dtype=np.float32)


# revision 17
# speedup vs baseline: 1.1114x; 1.0531x over previous
"""AUROC surrogate loss on 8 TRN2 NeuronCores.

Reference (for s = sigmoid(y_pred), pos = y_true==1, neg = y_true==0):
    total = sum_{i in pos, j in neg} relu(1 - (s_i - s_j));  loss = total/(P*Q)

Because s in [0,1], s_i - s_j in [-1,1], so 1 - (s_i - s_j) >= 0 ALWAYS and the
relu never clips.  The O(N^2) pairwise sum is therefore exactly linear:
    total = P*Q - Q*S_pos + P*S_neg   =>   loss = 1 - S_pos/P + S_neg/Q
with S_pos/S_neg the sums of s over the positive/negative class.  This turns
the problem into an O(N) streaming reduction (memory-bound), which is what the
device computes.

Sharding strategy (data-parallel over the positive/negative axes, per the
problem hint): each core gets a contiguous 1/8 shard; within the shard the
host packs positive elements into whole 128-wide partitions first, then
negatives (slack filled with -40, whose sigmoid ~ 4e-18 is negligible).  The
device kernel per core is a single fused pass:
    DMA in [17,128] -> ScalarE sigmoid with accum_out (per-partition sums)
    -> DMA out [17,1]
The per-class partition split point is host bookkeeping; the host all-reduces
the 8 cores' partial sums and applies the closed-form formula (P and Q are
label counts, computed host-side as part of the unshard).

Engine program notes (hard-won on real silicon):
  * the ACT memzero is the first ScalarE op so walrus hoists the sigmoid PWP
    table load (~1.3us) to t~0 where it overlaps the input DMA
  * the output DMA is issued by ScalarE itself, in order after the
    activation+accum -- no cross-engine semaphore hop
  * no completion wait on the output DMA: the NEFF's multi-microsecond
    engine-teardown sequence runs after the module body, giving the 68-byte
    transfer ample time to land before execution completes (validated over
    many runs); this keeps ~1us of HWDGE completion latency off the
    critical path.

Robustness: the axon/PJRT device session occasionally reports a transient
NRT_EXEC_UNIT_UNRECOVERABLE on a cold execution, and the failure is sticky
for the whole process.  kernel() retries in-process first, then falls back to
a fresh subprocess (new device session), which reliably recovers.
"""

import os
import subprocess
import sys
import tempfile
import time

import numpy as np

N = 16384
N_CORES = 8
SHARD = N // N_CORES  # 2048
PARTS = 17  # ceil(pos/128) + ceil(neg/128) <= 17 for any split of 2048
F = 128
PAD = np.float32(-40.0)  # sigmoid(-40) ~ 4.25e-18

_NC_CACHE = {}


def _make_fallthrough_block(bass):
    """Block whose exit emits each engine's end-barrier tail into that
    engine's last body bb (linear fall-through) instead of branching to a
    shared end bb -- saves the ~180ns always-taken branch on the critical
    engine between the output-DMA issue and the end barrier."""

    class _FallthroughBlock(bass.BassBlock):
        def __exit__(self, exc_type, exc_val, exc_tb):
            if exc_type is not None:
                return
            insts = self.bass._multi_engine_barrier_insts(list(self.bass.engines))
            by_engine = {}
            for i in insts:
                by_engine.setdefault(i.engine, []).append(i)
            done = set()
            for engine, last_body in self.last_body.items():
                et = engine.engine
                with self.bass.body(
                    last_body, parent=self.bass.cur_bb, allow_existing_parent=True
                ):
                    for i in by_engine.get(et, []):
                        self.bass.engines[i.engine].add_instruction(i)
                done.add(et)
            self.bass.switch_bb(self.end_bb)
            for et, eng in self.bass.engines.items():
                if et not in done:
                    for i in by_engine.get(et, []):
                        eng.add_instruction(i)

    return _FallthroughBlock


def build_nc():
    import concourse.bass as bass
    from concourse import mybir

    # Bass.__init__ registers four const tiles (gpsimd memsets, ~0.4us of dead
    # stores this kernel never reads -- the BIR verifier flags them as
    # reader-less) and closes with an all-engine barrier.  Skip both during
    # construction: the NEFF gets smaller and the measured critical path
    # starts at real work (worth ~2.2us on silicon).  Defensive: fall back to
    # the stock constructor on any error.
    ctor_kw = dict(
        num_devices=N_CORES, enable_partition_id=False, monotonic_sem_count=0
    )
    try:
        orig_barrier = bass.Bass.all_engine_barrier
        memset_owner = None
        for klass in bass.BassGpSimd.__mro__:
            if "memset" in vars(klass):
                memset_owner = klass
                break
        orig_memset = memset_owner.memset
        bass.Bass.all_engine_barrier = lambda self, **kw: None
        memset_owner.memset = lambda self, ap, c: None
        try:
            nc = bass.Bass(**ctor_kw)
        finally:
            bass.Bass.all_engine_barrier = orig_barrier
            memset_owner.memset = orig_memset
    except Exception:  # noqa: BLE001
        nc = bass.Bass(**ctor_kw)
    yp = nc.dram_tensor("yp_packed", [PARTS, F], mybir.dt.float32, kind="ExternalInput")
    out = nc.dram_tensor("psums", [PARTS, 1], mybir.dt.float32, kind="ExternalOutput")

    from contextlib import contextmanager

    @contextmanager
    def _block_ctx():
        blk = _make_fallthrough_block(bass)(nc, f"block_{nc.next_id()}")
        with blk:
            nc.cur_block = blk
            yield blk
        nc.cur_block = None

    with (
        nc.sbuf_tensor([PARTS, F], mybir.dt.float32) as ypt,
        nc.sbuf_tensor([PARTS, F], mybir.dt.float32) as s,
        nc.sbuf_tensor([PARTS, 1], mybir.dt.float32) as red,
        nc.sbuf_tensor([PARTS, 1], mybir.dt.float32) as bias,
        nc.semaphore() as sp,
        nc.semaphore() as so,
        _block_ctx() as block,
    ):

        @block.scalar
        def _(scalar):
            # first ACT op: walrus inserts the PWP table load before it,
            # overlapping the input DMA; also zeroes the sigmoid bias AP
            scalar.memzero(bias[:])
            scalar.drain()  # bias write retired before the activation reads it
            scalar.wait_ge(sp, 16)
            scalar.activation(
                out=s[:],
                in_=ypt[:],
                func=mybir.ActivationFunctionType.Sigmoid,
                bias=bias[:],
                accum_out=red[:],  # per-partition sum of sigmoid
            )
            # No retirement barrier before the DMA trigger: the accumulator
            # write retires with the READ_ACCUMULATOR machine op (~280ns engine
            # tail), while the HWDGE descriptor generation the trigger kicks
            # off takes ~550ns before the DMA engine reads red -- a structural
            # margin validated over hundreds of executions.  An explicit
            # drain/sem here costs ~0.9us (serializes the ACT pipeline tail
            # against descriptor generation).
            scalar.dma_start(out=out.ap(), in_=red[:]).then_inc(so, 16)

        @block.sync
        def _(sync):
            sync.dma_start(out=ypt[:], in_=yp.ap()).then_inc(sp, 16)

    return nc


def get_nc():
    if "nc" not in _NC_CACHE:
        _NC_CACHE["nc"] = build_nc()
    return _NC_CACHE["nc"]


def _pack_shard(yp_shard, yt_shard):
    """Pack one core's shard: positives fill whole partitions first (padded),
    then negatives (padded).  Returns (packed [PARTS,F] f32, n_pos_partitions).
    """
    pos = yp_shard[yt_shard == 1]
    neg = yp_shard[yt_shard == 0]
    pos_parts = (len(pos) + F - 1) // F
    packed = np.full((PARTS, F), PAD, dtype=np.float32)
    flat = packed.reshape(-1)
    flat[: len(pos)] = pos
    flat[pos_parts * F : pos_parts * F + len(neg)] = neg
    return packed, pos_parts


def _run_device(packed_list):
    """Run the SPMD kernel; returns [N_CORES, PARTS] f32 per-partition sums."""
    from concourse import bass_utils

    in_maps = [{"yp_packed": p} for p in packed_list]
    nc = get_nc()
    res = bass_utils.run_bass_kernel_spmd(nc, in_maps, core_ids=list(range(N_CORES)))
    return np.stack([np.asarray(r["psums"]).reshape(-1) for r in res.results])


_CHILD_CODE = """
import numpy as np, sys
sys.path.insert(0, {kdir!r})
import importlib.util
spec = importlib.util.spec_from_file_location("_kernel_child", {kfile!r})
mod = importlib.util.module_from_spec(spec)
spec.loader.exec_module(mod)
d = np.load(sys.argv[1])
psums = mod._run_device([d[f"p{{i}}"] for i in range({ncores})])
np.save(sys.argv[2], psums)
"""


def _run_device_subprocess(packed_list):
    kfile = os.path.abspath(__file__)
    code = _CHILD_CODE.format(kdir=os.path.dirname(kfile), kfile=kfile, ncores=N_CORES)
    with tempfile.TemporaryDirectory() as td:
        inp = os.path.join(td, "in.npz")
        outp = os.path.join(td, "out.npy")
        np.savez(inp, **{f"p{i}": p for i, p in enumerate(packed_list)})
        env = dict(os.environ)
        env.pop("JAX_PLATFORMS", None)
        subprocess.run(
            [sys.executable, "-c", code, inp, outp],
            check=True,
            env=env,
            timeout=900,
            capture_output=True,
        )
        return np.load(outp)


def kernel(y_pred, y_true):
    y_pred = np.asarray(y_pred, dtype=np.float32).reshape(N)
    y_true = np.asarray(y_true, dtype=np.int32).reshape(N)

    packed_list = []
    pos_parts = []
    for i in range(N_CORES):
        sl = slice(i * SHARD, (i + 1) * SHARD)
        packed, pp = _pack_shard(y_pred[sl], y_true[sl])
        packed_list.append(packed)
        pos_parts.append(pp)

    psums = None
    last_exc = None
    try:
        psums = _run_device(packed_list)
    except Exception as e:  # noqa: BLE001
        # A failed execution (e.g. transient NRT_EXEC_UNIT_UNRECOVERABLE) is
        # sticky for this process's device session -- recover via fresh
        # subprocesses (new session) instead of retrying in-process.
        last_exc = e
        for attempt in range(3):
            try:
                psums = _run_device_subprocess(packed_list)
                break
            except Exception as e2:  # noqa: BLE001
                last_exc = e2
                time.sleep(2.0 + 3.0 * attempt)
    if psums is None:
        raise last_exc

    s_pos = 0.0
    s_neg = 0.0
    for pp, row in zip(pos_parts, psums):
        row = row.astype(np.float64)
        s_pos += row[:pp].sum()
        s_neg += row[pp:].sum()

    p_cnt = float((y_true == 1).sum())
    q_cnt = float((y_true == 0).sum())
    if p_cnt * q_cnt <= 0:
        return np.array(0.0, dtype=np.float32)
    loss = 1.0 - s_pos / p_cnt + s_neg / q_cnt
    return np.array(loss, dtype=np.float32)
